# revision 27
# baseline (speedup 1.0000x reference)
"""MeshFC kernel for 8x TRN2 NeuronCores.

Computes: out = inputs @ w + biases, where
  w[i,o] = ||in_pos[i]-out_pos[o]|| - ||init_in_pos[i]-init_out_pos[o]||

Sharding: tensor-parallel on the output dim (8 x 1024 columns).

Per-core pipeline (all PE matmuls at 1 cycle/row):
  - weight gen via the difference form
        t = ||a0-b0||^2          u = t + D,  D = ||a-b||^2 - ||a0-b0||^2
        w = sqrt(u) - sqrt(t)
    t is computed from an fp16 hi/lo mantissa split (ah.bh + ah.bl + al.bh
    ~ 22-bit accuracy, err ~1e-5; single fp16/fp32r matmuls err at ~1e-3,
    swamping near-coincident pairs). D uses cancellation-free aug vectors
    (a, a-a0 paired with -2(b-b0), -2b0: every product is O(delta), so
    fp16 rounding costs only ~1e-5 on w). The three t terms (and for u
    also D) are stacked along the contraction dim into ONE K=128 matmul
    each, sharing one moving operand; zero rows blank out D for the t
    matmul. K is padded to 128 because the PE streams 1 cycle/row only
    for stationary K >= ~96 (2 cycles/row below - measured). Both sqrt
    args are clamped to >=0 on DVE (the ~2e-5 residual t error can go
    negative on near-coincident pairs -> NaN otherwise; an eps shift is
    NOT usable: any eps creates sqrt(eps)-scale w errors on pairs with
    dist0^2 < eps). The t-error otherwise cancels between the two sqrts
    since u adds D on top of the same computed t.
  - main [4096,2048]x[2048,1024] matmul in fp16, oh-phased (512-col
    halves) so weight-gen clamp/sqrt/sub work (spread over DVE, GpSimd
    and ScalarE) overlaps the matmul stream; x streams through an 8-deep
    SBUF pool.
  - biases are added on the host (they are zeros in this problem).
"""

import os
from contextlib import ExitStack

import numpy as np

NUM_IN, NUM_OUT, SD, BATCH = 2048, 8192, 5, 4096
N_CORES = 8
O_SHARD = NUM_OUT // N_CORES  # 1024
B_TILES = BATCH // 128  # 32
K_TILES = NUM_IN // 128  # 16
O_HALVES = O_SHARD // 512  # 2
EPS_T = 0.0  # t-clamp is on DVE; spare row 33 kept as a no-op

_CACHE = {}


def _split16(v):
    """hi/lo split at fp16 precision: v ~= hi + lo with both fp16."""
    hi = v.astype(np.float16)
    lo = (v - hi.astype(np.float64)).astype(np.float16)
    return hi, lo


def _build_bass(variant=""):
    import concourse.bass as bass  # noqa: F401
    import concourse.mybir as mybir
    from concourse import bacc
    from concourse.tile import TileContext

    fp32 = mybir.dt.float32
    fp32r = mybir.dt.float32r
    fp16 = mybir.dt.float16
    af = mybir.ActivationFunctionType

    nc = bacc.Bacc("TRN2", name="meshfc")

    xT = nc.dram_tensor("xT", [128, B_TILES * NUM_IN], fp16,
                        kind="ExternalInput")
    ab = nc.dram_tensor("ab", [128, 2 * NUM_IN + O_SHARD], fp16,
                        kind="ExternalInput")
    out = nc.dram_tensor("out", [BATCH, O_SHARD], fp32, kind="ExternalOutput")

    n_rep = 1
    for tok in variant.split(","):
        if tok.startswith("rep"):
            n_rep = int(tok[3:])

    with ExitStack() as ctx:
        tc = ctx.enter_context(TileContext(nc))
        const = ctx.enter_context(tc.tile_pool(name="const", bufs=1))
        wps = tmp = mps = opool = None
        if "nowgen" not in variant:
            wps = ctx.enter_context(tc.tile_pool(name="wps", bufs=2, space="PSUM"))
            tmp = ctx.enter_context(tc.tile_pool(name="tmp", bufs=2))
        if "nomm" not in variant:
            mps = ctx.enter_context(tc.tile_pool(name="mps", bufs=3, space="PSUM"))
            opool = ctx.enter_context(tc.tile_pool(name="op", bufs=3))

        # --- constants (K=128-stacked aug operands): ONE dma_start -- each
        # dma_start costs ~1us of descriptor generation on the issuing
        # sequencer, so fewer + contiguous beats many small ones ---
        ab_sb = const.tile([128, 2 * NUM_IN + O_SHARD], fp16, name="ab_sb")
        aU_sb = ab_sb[:, 0:NUM_IN]
        aT_sb = ab_sb[:, NUM_IN : 2 * NUM_IN]
        bU_sb = ab_sb[:, 2 * NUM_IN : 2 * NUM_IN + O_SHARD]
        # slice-DMAs issued first, split over the SP and ScalarE sequencers
        # so descriptor generation (~1us each) overlaps
        nc.sync.dma_start(out=bU_sb, in_=ab[:, 2 * NUM_IN :])
        nc.scalar.dma_start(out=aT_sb, in_=ab[:, NUM_IN : 2 * NUM_IN])
        nc.sync.dma_start(out=aU_sb, in_=ab[:, 0:NUM_IN])

        # PE warm-up: dependency-free dummy matmuls run while the input DMAs
        # land (PE is otherwise idle), so the DVFS pstate is at full clock
        # when the first real weight-gen matmul issues. Results are never
        # read; garbage operand values are harmless (worst case NaN psum).
        if "nowarm" not in variant:
            warm = const.tile([128, 512], fp16, name="warm")
            nc.vector.memzero(warm)
            wpsum = ctx.enter_context(
                tc.tile_pool(name="warmps", bufs=1, space="PSUM"))
            for _ in range(9):
                wp = wpsum.tile([128, 512], fp32, tag="wp", bufs=1)
                nc.tensor.matmul(wp, warm[:, 0:128], warm, start=True,
                                 stop=True)

        # x streamed per-bt through an 8-deep pool (32 KiB/part): 512 KB
        # DMAs with 4 KB descriptors interleave fairly behind the ab
        # descriptors; bigger chunks starve the weight-gen operand tail
        xpool = None
        if "nomm" not in variant:
            xpool = ctx.enter_context(tc.tile_pool(name="xp", bufs=8))

        # per-half weight blocks (separate tiles so tile-level deps give
        # oh-granular pipelining even if subtile tracking is conservative)
        whs = [
            const.tile([128, K_TILES, 512], fp16, name=f"w{oh}")
            for oh in range(O_HALVES)
        ]

        for _rep in range(n_rep):
            _build_body(nc, variant, af, wps, tmp, mps, opool, xpool,
                        aU_sb, aT_sb, bU_sb, whs, xT, out, fp32, fp16)

    nc.finalize()
    return nc


def _build_body(nc, variant, af, wps, tmp, mps, opool, xpool,
                aU_sb, aT_sb, bU_sb, whs, xT, out, fp32, fp16):
    # --- weight generation: w = sqrt(u) - sqrt(t) ---
    if "nowgen" not in variant:
        for oh in range(O_HALVES):
            osl = slice(oh * 512, (oh + 1) * 512)
            for kt in range(K_TILES):
                ksl = slice(kt * 128, (kt + 1) * 128)
                psT = wps.tile([128, 512], fp32, tag="psT", bufs=2)
                psU = wps.tile([128, 512], fp32, tag="psU", bufs=2)
                nc.tensor.matmul(psT, aT_sb[:, ksl], bU_sb[:, osl],
                                 start=True, stop=True)
                nc.tensor.matmul(psU, aU_sb[:, ksl], bU_sb[:, osl],
                                 start=True, stop=True)
                u = tmp.tile([128, 512], fp32, tag="u", bufs=3)
                sI = tmp.tile([128, 512], fp32, tag="sI", bufs=3)
                # clamps to >=0 double as the PSUM->SBUF moves; both on DVE
                # (GPSIMD cannot read PSUM, and ScalarE-side clamps delay
                # PSUM-bank release -> PE backpressure). ScalarE does the two
                # sqrts, GpSimd the SBUF-only sub: ~1.3us/tile chain cadence
                nc.vector.tensor_scalar_max(sI, psT, 0.0)
                nc.vector.tensor_scalar_max(u, psU, 0.0)
                nc.scalar.sqrt(sI, sI)
                nc.scalar.sqrt(u, u)
                nc.gpsimd.tensor_sub(whs[oh][:, kt, :], u, sI)

    # --- main matmul, oh-phased: out[b, osl] = x[b, :] @ w[:, osl] ---
    if "nomm" in variant:
        return
    for oh in range(O_HALVES):
        osl = slice(oh * 512, (oh + 1) * 512)
        for bt in range(B_TILES):
            xt = xpool.tile([128, NUM_IN], fp16, tag="xt", bufs=8)
            nc.sync.dma_start(
                out=xt, in_=xT[:, bt * NUM_IN : (bt + 1) * NUM_IN])
            ps = mps.tile([128, 512], fp32, tag="ps", bufs=3)
            for kt in range(K_TILES):
                nc.tensor.matmul(
                    ps,
                    xt[:, kt * 128 : (kt + 1) * 128],
                    whs[oh][:, kt, :],
                    start=(kt == 0),
                    stop=(kt == K_TILES - 1),
                )
            ot = opool.tile([128, 512], fp32, tag="ot", bufs=3)
            nc.scalar.copy(ot, ps)
            # issue the out-DMA from ScalarE: it just produced ot (no
            # cross-engine sem hop) and the sync sequencer is loaded with
            # x-tile dma_starts -- SP backlog otherwise delays the final
            # out-DMA past PE-end
            nc.scalar.dma_start(out=out[bt * 128 : (bt + 1) * 128, osl],
                                in_=ot)


def _prep_inputs(inputs, init_in_pos, init_out_pos, in_pos, out_pos, biases):
    x = np.asarray(inputs, dtype=np.float32)
    a = np.asarray(in_pos, dtype=np.float64).reshape(NUM_IN, SD)
    a0 = np.asarray(init_in_pos, dtype=np.float64).reshape(NUM_IN, SD)
    b = np.asarray(out_pos, dtype=np.float64).reshape(NUM_OUT, SD)
    b0 = np.asarray(init_out_pos, dtype=np.float64).reshape(NUM_OUT, SD)
    bias = np.asarray(biases, dtype=np.float32).reshape(NUM_OUT)

    # xT[d, bt*2048 + kt*128 + b'] = x[bt*128+b', kt*128+d]
    xT = np.ascontiguousarray(
        x.reshape(B_TILES, 128, K_TILES, 128).transpose(3, 0, 2, 1)
        .astype(np.float16)
    ).reshape(128, B_TILES * NUM_IN)

    da, db = a - a0, b - b0
    Sa = (a * a).sum(1) - (a0 * a0).sum(1)
    Sb = (b * b).sum(1) - (b0 * b0).sum(1)

    ones_i = np.ones(NUM_IN)
    ones_o = np.ones(NUM_OUT)
    # D[i,o] = a.(-2db) + da.(-2b0) + Sa*1 + 1*Sb  = dist^2 - dist0^2
    aD = np.concatenate([a.T, da.T, Sa[None, :], ones_i[None, :]], 0)
    bD_full = np.concatenate([-2.0 * db.T, -2.0 * b0.T, ones_o[None, :],
                              Sb[None, :]], 0)
    # t[i,o] = a0.(-2b0) + |a0|^2*1 + 1*|b0|^2 = dist0^2,
    # via fp16 hi/lo split: t = ah.bh + ah.bl + al.bh
    aT7 = np.concatenate([a0.T, (a0 * a0).sum(1)[None, :], ones_i[None, :]], 0)
    bT7_full = np.concatenate([-2.0 * b0.T, ones_o[None, :],
                               (b0 * b0).sum(1)[None, :]], 0)
    ah, al = _split16(aT7)
    bh_full, bl_full = _split16(bT7_full)
    # K=128 stacks sharing one moving operand bU = [bh|bl|bh|bD|0]:
    #   u-matmul stationary [ah|ah|al|aD|0] -> t + D
    #   t-matmul stationary [ah|ah|al| 0|0] -> t
    zpad = np.zeros((128 - 33, NUM_IN), np.float16)
    z12 = np.zeros((12, NUM_IN), np.float16)
    aUs = np.concatenate([ah, ah, al, aD.astype(np.float16), zpad], 0)
    aTs = np.concatenate([ah, ah, al, z12, zpad], 0)
    zpad_o = np.zeros((128 - 33, NUM_OUT), np.float16)
    bUs_full = np.concatenate([bh_full, bl_full, bh_full,
                               bD_full.astype(np.float16), zpad_o], 0)
    # spare row 33 can shift t by EPS_T without touching u; unused
    # (EPS_T=0) since the device clamps t on DVE anyway
    aTs[33, :] = EPS_T
    bUs_full[33, :] = 1.0

    in_maps = []
    for c in range(N_CORES):
        sl = slice(c * O_SHARD, (c + 1) * O_SHARD)
        ab = np.ascontiguousarray(
            np.concatenate([aUs, aTs, bUs_full[:, sl]], axis=1))
        in_maps.append({"xT": xT, "ab": ab})
    return in_maps, bias


def _run(in_maps, trace=False):
    from concourse.bass_utils import run_bass_kernel_spmd

    variant = os.environ.get("MESHFC_VARIANT", "")
    key = ("nc", variant)
    if key not in _CACHE:
        _CACHE[key] = _build_bass(variant)
    nc = _CACHE[key]
    res = run_bass_kernel_spmd(
        nc, in_maps, core_ids=list(range(N_CORES)), trace=trace
    )
    outs = [r["out"] for r in res.results]
    return np.concatenate(outs, axis=1), res


def kernel(**inputs) -> np.ndarray:
    in_maps, bias = _prep_inputs(**inputs)
    # no tracing here: the NTFF profile hook may be absent in the grading
    # environment, and trace=True would then fail the run
    out, _ = _run(in_maps, trace=False)
    if bias.any():
        out = out + bias[None, :]
    return out


# revision 30
# speedup vs baseline: 1.0289x; 1.0289x over previous
"""MeshFC kernel for 8x TRN2 NeuronCores.

Computes: out = inputs @ w + biases, where
  w[i,o] = ||in_pos[i]-out_pos[o]|| - ||init_in_pos[i]-init_out_pos[o]||

Sharding: tensor-parallel on the output dim (8 x 1024 columns).

Per-core pipeline (all PE matmuls at 1 cycle/row):
  - weight gen via the difference form
        t = ||a0-b0||^2          u = t + D,  D = ||a-b||^2 - ||a0-b0||^2
        w = sqrt(u) - sqrt(t)
    t is computed from an fp16 hi/lo mantissa split (ah.bh + ah.bl + al.bh
    ~ 22-bit accuracy, err ~1e-5; single fp16/fp32r matmuls err at ~1e-3,
    swamping near-coincident pairs). D uses cancellation-free aug vectors
    (a, a-a0 paired with -2(b-b0), -2b0: every product is O(delta), so
    fp16 rounding costs only ~1e-5 on w). The three t terms (and for u
    also D) are stacked along the contraction dim into ONE K=128 matmul
    each, sharing one moving operand; zero rows blank out D for the t
    matmul. K is padded to 128 because the PE streams 1 cycle/row only
    for stationary K >= ~96 (2 cycles/row below - measured). Both sqrt
    args are clamped to >=0 on DVE (the ~2e-5 residual t error can go
    negative on near-coincident pairs -> NaN otherwise; an eps shift is
    NOT usable: any eps creates sqrt(eps)-scale w errors on pairs with
    dist0^2 < eps). The t-error otherwise cancels between the two sqrts
    since u adds D on top of the same computed t.
  - main [4096,2048]x[2048,1024] matmul in fp16, oh-phased (512-col
    halves) so weight-gen clamp/sqrt/sub work (spread over DVE, GpSimd
    and ScalarE) overlaps the matmul stream; x streams through an 8-deep
    SBUF pool.
  - biases are added on the host (they are zeros in this problem).
"""

import os
from contextlib import ExitStack

import numpy as np

NUM_IN, NUM_OUT, SD, BATCH = 2048, 8192, 5, 4096
N_CORES = 8
O_SHARD = NUM_OUT // N_CORES  # 1024
B_TILES = BATCH // 128  # 32
K_TILES = NUM_IN // 128  # 16
O_HALVES = O_SHARD // 512  # 2
EPS_T = 0.0  # t-clamp is on DVE; spare row 33 kept as a no-op

_CACHE = {}


def _split16(v):
    """hi/lo split at fp16 precision: v ~= hi + lo with both fp16."""
    hi = v.astype(np.float16)
    lo = (v - hi.astype(np.float64)).astype(np.float16)
    return hi, lo


def _build_bass(variant=""):
    import concourse.bass as bass  # noqa: F401
    import concourse.mybir as mybir
    from concourse import bacc
    from concourse.tile import TileContext

    fp32 = mybir.dt.float32
    fp32r = mybir.dt.float32r
    fp16 = mybir.dt.float16
    af = mybir.ActivationFunctionType

    nc = bacc.Bacc("TRN2", name="meshfc")

    xT = nc.dram_tensor("xT", [128, B_TILES * NUM_IN], fp16,
                        kind="ExternalInput")
    ab = nc.dram_tensor("ab", [128, 2 * NUM_IN + O_SHARD], fp16,
                        kind="ExternalInput")
    out = nc.dram_tensor("out", [BATCH, O_SHARD], fp32, kind="ExternalOutput")

    n_rep = 1
    for tok in variant.split(","):
        if tok.startswith("rep"):
            n_rep = int(tok[3:])

    with ExitStack() as ctx:
        tc = ctx.enter_context(TileContext(nc))
        const = ctx.enter_context(tc.tile_pool(name="const", bufs=1))
        wps = tmp = mps = opool = None
        if "nowgen" not in variant:
            wps = ctx.enter_context(tc.tile_pool(name="wps", bufs=2, space="PSUM"))
            tmp = ctx.enter_context(tc.tile_pool(name="tmp", bufs=2))
        if "nomm" not in variant:
            mps = ctx.enter_context(tc.tile_pool(name="mps", bufs=3, space="PSUM"))
            opool = ctx.enter_context(tc.tile_pool(name="op", bufs=3))

        # --- constants (K=128-stacked aug operands): ONE dma_start -- each
        # dma_start costs ~1us of descriptor generation on the issuing
        # sequencer, so fewer + contiguous beats many small ones ---
        ab_sb = const.tile([128, 2 * NUM_IN + O_SHARD], fp16, name="ab_sb")
        aU_sb = ab_sb[:, 0:NUM_IN]
        aT_sb = ab_sb[:, NUM_IN : 2 * NUM_IN]
        bU_sb = ab_sb[:, 2 * NUM_IN : 2 * NUM_IN + O_SHARD]
        # slice-DMAs issued first, split over the SP and ScalarE sequencers
        # so descriptor generation (~1us each) overlaps
        nc.sync.dma_start(out=bU_sb, in_=ab[:, 2 * NUM_IN :])
        nc.scalar.dma_start(out=aT_sb, in_=ab[:, NUM_IN : 2 * NUM_IN])
        nc.sync.dma_start(out=aU_sb, in_=ab[:, 0:NUM_IN])

        # PE warm-up: dependency-free dummy matmuls run while the input DMAs
        # land (PE is otherwise idle), so the DVFS pstate is at full clock
        # when the first real weight-gen matmul issues. Results are never
        # read; garbage operand values are harmless (worst case NaN psum).
        if "nowarm" not in variant:
            warm = const.tile([128, 512], fp16, name="warm")
            nc.vector.memzero(warm)
            wpsum = ctx.enter_context(
                tc.tile_pool(name="warmps", bufs=1, space="PSUM"))
            for _ in range(9):
                wp = wpsum.tile([128, 512], fp32, tag="wp", bufs=1)
                nc.tensor.matmul(wp, warm[:, 0:128], warm, start=True,
                                 stop=True)

        # x streamed per-bt through an 8-deep pool (32 KiB/part): 512 KB
        # DMAs with 4 KB descriptors interleave fairly behind the ab
        # descriptors; bigger chunks starve the weight-gen operand tail
        xpool = None
        if "nomm" not in variant:
            xpool = ctx.enter_context(tc.tile_pool(name="xp", bufs=8))

        # per-half weight blocks (separate tiles so tile-level deps give
        # oh-granular pipelining even if subtile tracking is conservative)
        whs = [
            const.tile([128, K_TILES, 512], fp16, name=f"w{oh}")
            for oh in range(O_HALVES)
        ]

        for _rep in range(n_rep):
            _build_body(nc, variant, af, wps, tmp, mps, opool, xpool,
                        aU_sb, aT_sb, bU_sb, whs, xT, out, fp32, fp16)

    nc.finalize()
    return nc


def _build_body(nc, variant, af, wps, tmp, mps, opool, xpool,
                aU_sb, aT_sb, bU_sb, whs, xT, out, fp32, fp16):
    # --- weight generation: w = sqrt(u) - sqrt(t) ---
    if "nowgen" not in variant:
        for oh in range(O_HALVES):
            osl = slice(oh * 512, (oh + 1) * 512)
            for kt in range(K_TILES):
                ksl = slice(kt * 128, (kt + 1) * 128)
                psT = wps.tile([128, 512], fp32, tag="psT", bufs=2)
                psU = wps.tile([128, 512], fp32, tag="psU", bufs=2)
                nc.tensor.matmul(psT, aT_sb[:, ksl], bU_sb[:, osl],
                                 start=True, stop=True)
                nc.tensor.matmul(psU, aU_sb[:, ksl], bU_sb[:, osl],
                                 start=True, stop=True)
                u = tmp.tile([128, 512], fp32, tag="u", bufs=3)
                sI = tmp.tile([128, 512], fp32, tag="sI", bufs=3)
                # clamps to >=0 double as the PSUM->SBUF moves; both on DVE
                # (GPSIMD cannot read PSUM, and ScalarE-side clamps delay
                # PSUM-bank release -> PE backpressure). ScalarE does the two
                # sqrts, GpSimd the SBUF-only sub: ~1.3us/tile chain cadence
                nc.vector.tensor_scalar_max(sI, psT, 0.0)
                nc.vector.tensor_scalar_max(u, psU, 0.0)
                nc.scalar.sqrt(sI, sI)
                nc.scalar.sqrt(u, u)
                nc.gpsimd.tensor_sub(whs[oh][:, kt, :], u, sI)

    # --- main matmul, oh-phased: out[b, osl] = x[b, :] @ w[:, osl] ---
    if "nomm" in variant:
        return
    for oh in range(O_HALVES):
        osl = slice(oh * 512, (oh + 1) * 512)
        for bt in range(B_TILES):
            xt = xpool.tile([128, NUM_IN], fp16, tag="xt", bufs=8)
            nc.sync.dma_start(
                out=xt, in_=xT[:, bt * NUM_IN : (bt + 1) * NUM_IN])
            ps = mps.tile([128, 512], fp32, tag="ps", bufs=3)
            for kt in range(K_TILES):
                nc.tensor.matmul(
                    ps,
                    xt[:, kt * 128 : (kt + 1) * 128],
                    whs[oh][:, kt, :],
                    start=(kt == 0),
                    stop=(kt == K_TILES - 1),
                )
            ot = opool.tile([128, 512], fp32, tag="ot", bufs=3)
            nc.scalar.copy(ot, ps)
            nc.sync.dma_start(out=out[bt * 128 : (bt + 1) * 128, osl], in_=ot)


def _prep_inputs(inputs, init_in_pos, init_out_pos, in_pos, out_pos, biases):
    x = np.asarray(inputs, dtype=np.float32)
    a = np.asarray(in_pos, dtype=np.float64).reshape(NUM_IN, SD)
    a0 = np.asarray(init_in_pos, dtype=np.float64).reshape(NUM_IN, SD)
    b = np.asarray(out_pos, dtype=np.float64).reshape(NUM_OUT, SD)
    b0 = np.asarray(init_out_pos, dtype=np.float64).reshape(NUM_OUT, SD)
    bias = np.asarray(biases, dtype=np.float32).reshape(NUM_OUT)

    # xT[d, bt*2048 + kt*128 + b'] = x[bt*128+b', kt*128+d]
    xT = np.ascontiguousarray(
        x.reshape(B_TILES, 128, K_TILES, 128).transpose(3, 0, 2, 1)
        .astype(np.float16)
    ).reshape(128, B_TILES * NUM_IN)

    da, db = a - a0, b - b0
    Sa = (a * a).sum(1) - (a0 * a0).sum(1)
    Sb = (b * b).sum(1) - (b0 * b0).sum(1)

    ones_i = np.ones(NUM_IN)
    ones_o = np.ones(NUM_OUT)
    # D[i,o] = a.(-2db) + da.(-2b0) + Sa*1 + 1*Sb  = dist^2 - dist0^2
    aD = np.concatenate([a.T, da.T, Sa[None, :], ones_i[None, :]], 0)
    bD_full = np.concatenate([-2.0 * db.T, -2.0 * b0.T, ones_o[None, :],
                              Sb[None, :]], 0)
    # t[i,o] = a0.(-2b0) + |a0|^2*1 + 1*|b0|^2 = dist0^2,
    # via fp16 hi/lo split: t = ah.bh + ah.bl + al.bh
    aT7 = np.concatenate([a0.T, (a0 * a0).sum(1)[None, :], ones_i[None, :]], 0)
    bT7_full = np.concatenate([-2.0 * b0.T, ones_o[None, :],
                               (b0 * b0).sum(1)[None, :]], 0)
    ah, al = _split16(aT7)
    bh_full, bl_full = _split16(bT7_full)
    # K=128 stacks sharing one moving operand bU = [bh|bl|bh|bD|0]:
    #   u-matmul stationary [ah|ah|al|aD|0] -> t + D
    #   t-matmul stationary [ah|ah|al| 0|0] -> t
    zpad = np.zeros((128 - 33, NUM_IN), np.float16)
    z12 = np.zeros((12, NUM_IN), np.float16)
    aUs = np.concatenate([ah, ah, al, aD.astype(np.float16), zpad], 0)
    aTs = np.concatenate([ah, ah, al, z12, zpad], 0)
    zpad_o = np.zeros((128 - 33, NUM_OUT), np.float16)
    bUs_full = np.concatenate([bh_full, bl_full, bh_full,
                               bD_full.astype(np.float16), zpad_o], 0)
    # spare row 33 can shift t by EPS_T without touching u; unused
    # (EPS_T=0) since the device clamps t on DVE anyway
    aTs[33, :] = EPS_T
    bUs_full[33, :] = 1.0

    in_maps = []
    for c in range(N_CORES):
        sl = slice(c * O_SHARD, (c + 1) * O_SHARD)
        ab = np.ascontiguousarray(
            np.concatenate([aUs, aTs, bUs_full[:, sl]], axis=1))
        in_maps.append({"xT": xT, "ab": ab})
    return in_maps, bias


def _run(in_maps, trace=False):
    from concourse.bass_utils import run_bass_kernel_spmd

    variant = os.environ.get("MESHFC_VARIANT", "")
    key = ("nc", variant)
    if key not in _CACHE:
        _CACHE[key] = _build_bass(variant)
    nc = _CACHE[key]
    res = run_bass_kernel_spmd(
        nc, in_maps, core_ids=list(range(N_CORES)), trace=trace
    )
    outs = [r["out"] for r in res.results]
    return np.concatenate(outs, axis=1), res


def kernel(**inputs) -> np.ndarray:
    in_maps, bias = _prep_inputs(**inputs)
    # no tracing here: the NTFF profile hook may be absent in the grading
    # environment, and trace=True would then fail the run
    out, _ = _run(in_maps, trace=False)
    if bias.any():
        out = out + bias[None, :]
    return out
